# revision 18
# baseline (speedup 1.0000x reference)
"""BiLSTM tagger on 8 Trainium2 NeuronCores.

Reference computation (S=512, B=64, V=100000, E=128, H=256, T=64):
    x  = emb[inputs]                                  # [S,B,E]
    hf = LSTM_f(x);  hb = reverse(LSTM_b(reverse(x))) # [S,B,H] each
    out = concat(hf,hb) @ W_out.T + b_out             # [S,B,T]

Sharding: data-parallel over batch *and* direction.  Cores 0-3 run the
forward LSTM on batch slices of 16; cores 4-7 run the backward LSTM
(time-reversed indices) on the same batch slices.  Each core computes a
partial output projection with its direction's half of W_out; the host
sums fwd+bwd partials and adds b_out.

Per-core device pipeline (all compute on the NeuronCore):
  1. indirect-DMA gather of embedding rows (bf16 table) -> [tok,E] tiles
  2. PE transpose -> xT [E, 8192]
  3. x-projection GEMM (W_ih, bf16) + bias -> xpT [128, S*128] bf16 in SBUF
     (gate rows permuted to chunk order [i0,i1,f0,f1,o0,o1,g0,g1])
  4. 512-step LSTM scan: per step 16 matmuls (W_hh stationary, bf16,
     fast-weight-load) into PSUM, gates on scalar/vector engines in a
     [128, 8*16] packed layout, fp32 cell state, bf16 h
  5. output projection GEMM from saved h history, partials DMA'd out
"""

import sys

for _p in ("/opt/trn_rl_repo",):
    if _p not in sys.path:
        sys.path.insert(0, _p)

import numpy as np
import ml_dtypes

import concourse.bass as bass
import concourse.bacc as bacc
import concourse.mybir as mybir
import concourse.tile as tile
from concourse.bass import ts
from concourse.bass_utils import run_bass_kernel_spmd
from concourse.masks import make_identity

BF16 = mybir.dt.bfloat16
F32 = mybir.dt.float32
AF = mybir.ActivationFunctionType

S, B, V, E, H, T = 512, 64, 100000, 128, 256, 64
NCORES = 8
BL = B // (NCORES // 2)      # 16 batch per core
NTOK = S * BL                # 8192 tokens per core
G4H = 4 * H                  # 1024 gate rows
NCH = G4H // 128             # 8 gate-row chunks
NJT = NTOK // 128            # 64 gather tiles
NSL = NTOK // 512            # 16 GEMM slices

# gate-row permutation: torch order i,f,g,o -> chunk order i,f,o,g so the
# sigmoid gates (i,f,o) are contiguous in the packed layout
_PERM = np.concatenate(
    [np.arange(0, 2 * H), np.arange(3 * H, 4 * H), np.arange(2 * H, 3 * H)]
)


def build_program(n_steps: int = S) -> bass.Bass:
    NTOK = n_steps * BL
    NJT = NTOK // 128
    NSL = NTOK // 512

    nc = bacc.Bacc("TRN2", target_bir_lowering=False, debug=False)

    emb_d = nc.declare_dram_parameter("emb", [V, E], BF16, isOutput=False)
    idx_d = nc.declare_dram_parameter("idx", [128, NJT], mybir.dt.int32, isOutput=False)
    wih_d = nc.declare_dram_parameter("wih", [128, G4H], BF16, isOutput=False)
    whh_d = nc.declare_dram_parameter("whh", [H, G4H], BF16, isOutput=False)
    bias_d = nc.declare_dram_parameter("bias", [128, NCH], F32, isOutput=False)
    wout_d = nc.declare_dram_parameter("wout", [H, T], BF16, isOutput=False)
    out_d = nc.declare_dram_parameter("out", [T, NTOK], F32, isOutput=True)

    with tile.TileContext(nc) as tc:
        with (
            tc.tile_pool(name="persist", bufs=1) as pp,
            tc.tile_pool(name="tpsum", bufs=1, space="PSUM") as tpp,
            tc.tile_pool(name="gpsum", bufs=2, space="PSUM") as gpp,
            tc.tile_pool(name="spsumA", bufs=2, space="PSUM") as spA,
            tc.tile_pool(name="spsumB", bufs=2, space="PSUM") as spB,
            tc.tile_pool(name="opsum", bufs=1, space="PSUM") as opp,
            tc.tile_pool(name="gwork", bufs=3) as gwp,
            tc.tile_pool(name="swork", bufs=3) as swp,
        ):
            # ---- persistent SBUF tensors ----
            idx_sb = pp.tile([128, NJT], mybir.dt.int32, tag="idx")
            wih_sb = pp.tile([128, G4H], BF16, tag="wih")
            whh0_sb = pp.tile([128, G4H], BF16, tag="whh0")
            whh1_sb = pp.tile([128, G4H], BF16, tag="whh1")
            bias_sb = pp.tile([128, NCH], F32, tag="bias")
            wout_sb = pp.tile([128, 2 * T], BF16, tag="wout")
            ident = pp.tile([128, 128], BF16, tag="ident")
            xpT = pp.tile([128, n_steps * 128], BF16, tag="xpT")
            hsA = pp.tile([128, (n_steps + 1) * BL], BF16, tag="hsA")
            hsB = pp.tile([128, (n_steps + 1) * BL], BF16, tag="hsB")
            # per-stream [tanh(g) | c] pair tile; c lives in the high half
            gcA = pp.tile([128, BL * 2], F32, tag="gcA")
            gcB = pp.tile([128, BL * 2], F32, tag="gcB")
            # flat gather buffer: every gather writes a virgin region (the
            # dynamic-DMA descriptor has a single sem-wait slot, so a gather
            # may carry at most one dependency).  Each [tok,E] region is then
            # transposed in place (PE transpose -> PSUM -> DVE copy back) to
            # [E,tok], and the GEMM reads the buffer directly as rhs.
            xgb = pp.tile([128, NTOK], BF16, tag="xgb")

            # ---- load constants ----
            # idx goes through gpsimd's SWDGE queue (same queue as the
            # indirect gathers) so the gather needs no cross-queue wait:
            # the dynamic-DMA descriptor format only fits one sem wait.
            nc.gpsimd.dma_start(out=idx_sb[:], in_=idx_d[:])
            nc.sync.dma_start(out=wih_sb[:], in_=wih_d[:])
            nc.sync.dma_start(out=whh0_sb[:], in_=whh_d[0:128, :])
            nc.sync.dma_start(out=whh1_sb[:], in_=whh_d[128:256, :])
            nc.sync.dma_start(out=bias_sb[:], in_=bias_d[:])
            nc.sync.dma_start(out=wout_sb[:, 0:T], in_=wout_d[0:128, :])
            nc.sync.dma_start(out=wout_sb[:, T : 2 * T], in_=wout_d[128:256, :])
            make_identity(nc, ident[:])

            nc.gpsimd.memset(hsA[:, 0:BL], 0.0)
            nc.gpsimd.memset(hsB[:, 0:BL], 0.0)
            nc.gpsimd.memset(gcA[:], 0.0)
            nc.gpsimd.memset(gcB[:], 0.0)

            # ---- gather + in-place transpose + x-projection GEMM ----
            xp4 = xpT[:].rearrange("p (t c b) -> p t c b", c=NCH, b=BL)
            for j in range(NJT):
                nc.gpsimd.indirect_dma_start(
                    out=xgb[:, ts(j, 128)],
                    out_offset=None,
                    in_=emb_d[:],
                    in_offset=bass.IndirectOffsetOnAxis(
                        ap=idx_sb[:, j : j + 1], axis=0
                    ),
                )
                tp = tpp.tile([128, 128], BF16, tag="tp")
                nc.tensor.transpose(tp[:], xgb[:, ts(j, 128)], ident[:])
                # scalar-engine copy: the DVE copy lowers to the one-wait-slot
                # S4D4_TR encoding, and this instruction needs two waits
                # (PSUM ready + gather WAW on the region)
                nc.scalar.copy(out=xgb[:, ts(j, 128)], in_=tp[:])
            for s in range(NSL):
                for c in range(NCH):
                    pg = gpp.tile([128, 512], F32, tag="pg")
                    nc.tensor.matmul(
                        pg[:],
                        lhsT=wih_sb[:, ts(c, 128)],
                        rhs=xgb[:, ts(s, 512)],
                        start=True,
                        stop=True,
                    )
                    # copy+bias into the packed [t, c, b] layout
                    nc.scalar.activation(
                        out=xp4[:, s * 32 : (s + 1) * 32, c, :],
                        in_=pg[:].rearrange("p (t b) -> p t b", b=BL),
                        func=AF.Identity,
                        bias=bias_sb[:, c : c + 1],
                        scale=1.0,
                    )

            # ---- LSTM scan: two interleaved streams (batch 0:8 / 8:16) ----
            # Per stream and step:
            #   PE:  8 identity-matmuls accumulate xp into PSUM (start=True),
            #        then 16 W_hh matmuls (two H k-tiles x 8 gate chunks)
            #   ACT: sigmoid(i,f,o) and tanh(g) straight from PSUM
            #   DVE: [i|f] * [tanh_g|c] in one op, c = t12a+t12b,
            #        h = o * tanh(c) written as one [k0|k1] bf16 slice
            # The two streams have no data dependence, so the scheduler
            # overlaps stream A's gate chain with stream B's matmuls.
            HB = BL // 2  # 8 batch per stream
            for t in range(n_steps):
                for strm in range(2):
                    ps = (spA if strm == 0 else spB).tile(
                        [128, NCH * HB], F32, tag=f"ps{strm}"
                    )
                    hsv = hsA if strm == 0 else hsB
                    gcv = gcA if strm == 0 else gcB
                    for c in range(NCH):
                        nc.tensor.matmul(
                            ps[:, ts(c, HB)],
                            lhsT=ident[:],
                            rhs=xp4[:, t, c, strm * HB : (strm + 1) * HB],
                            start=True,
                            stop=False,
                        )
                        nc.tensor.matmul(
                            ps[:, ts(c, HB)],
                            lhsT=whh0_sb[:, ts(c, 128)],
                            rhs=hsv[:, t * 2 * HB : t * 2 * HB + HB],
                            start=False,
                            stop=False,
                        )
                        nc.tensor.matmul(
                            ps[:, ts(c, HB)],
                            lhsT=whh1_sb[:, ts(c, 128)],
                            rhs=hsv[:, t * 2 * HB + HB : (t + 1) * 2 * HB],
                            start=False,
                            stop=True,
                        )
                    gs = gwp.tile([128, 6 * HB], F32, tag=f"gs{strm}")
                    nc.scalar.activation(
                        gs[:, 0 : 6 * HB], ps[:, 0 : 6 * HB], AF.Sigmoid
                    )
                    nc.scalar.activation(
                        gcv[:, 0 : 2 * HB], ps[:, 6 * HB : 8 * HB], AF.Tanh
                    )
                    t12 = swp.tile([128, 4 * HB], F32, tag=f"t12{strm}")
                    nc.vector.tensor_mul(t12[:], gs[:, 0 : 4 * HB], gcv[:])
                    nc.vector.tensor_add(
                        gcv[:, 2 * HB : 4 * HB],
                        t12[:, 0 : 2 * HB],
                        t12[:, 2 * HB : 4 * HB],
                    )
                    th = swp.tile([128, 2 * HB], F32, tag=f"th{strm}")
                    nc.scalar.activation(th[:], gcv[:, 2 * HB : 4 * HB], AF.Tanh)
                    nc.vector.tensor_mul(
                        hsv[:, (t + 1) * 2 * HB : (t + 2) * 2 * HB],
                        gs[:, 4 * HB : 6 * HB],
                        th[:],
                    )

            # ---- output projection ----
            # tokens laid out [strm, t, b] in out_d so each matmul's PSUM
            # maps to a contiguous DRAM block
            for strm in range(2):
                hsv = hsA if strm == 0 else hsB
                hs4 = hsv[:].rearrange("p (t k b) -> p t k b", k=2, b=HB)
                for s in range(n_steps * HB // 512):
                    po = opp.tile([T, 512], F32, tag="po")
                    tsl = slice(1 + s * 64, 1 + (s + 1) * 64)
                    nc.tensor.matmul(
                        po[:],
                        lhsT=wout_sb[:, 0:T],
                        rhs=hs4[:, tsl, 0, :],
                        start=True,
                        stop=False,
                    )
                    nc.tensor.matmul(
                        po[:],
                        lhsT=wout_sb[:, T : 2 * T],
                        rhs=hs4[:, tsl, 1, :],
                        start=False,
                        stop=True,
                    )
                    og = swp.tile([T, 512], F32, tag="og")
                    nc.vector.tensor_copy(out=og[:], in_=po[:])
                    nc.sync.dma_start(
                        out=out_d[:, strm * (n_steps * HB) + s * 512 :
                                  strm * (n_steps * HB) + (s + 1) * 512],
                        in_=og[:],
                    )

    nc.compile()
    return nc


_PROGRAM_CACHE: list = []


def _get_program() -> bass.Bass:
    if not _PROGRAM_CACHE:
        _PROGRAM_CACHE.append(build_program())
    return _PROGRAM_CACHE[0]


def _core_inputs(core, inputs_i32, emb_bf, weights):
    fwd = core < 4
    bsl = slice((core % 4) * BL, (core % 4) * BL + BL)
    W_ih, W_hh, b_ih, b_hh, W_out = weights[0 if fwd else 1]

    ids = inputs_i32[:, bsl]
    if not fwd:
        ids = ids[::-1]
    idx_t = np.ascontiguousarray(ids.reshape(NJT, 128).T).astype(np.int32)

    Wihp = W_ih[_PERM]                       # [4H, E]
    wih = np.ascontiguousarray(Wihp.T).astype(ml_dtypes.bfloat16)  # [E, 4H]
    Whhp = W_hh[_PERM]                       # [4H, H]
    whh = np.ascontiguousarray(Whhp.T).astype(ml_dtypes.bfloat16)  # [H, 4H]
    bp = (b_ih + b_hh)[_PERM].astype(np.float32)
    bias = np.ascontiguousarray(bp.reshape(NCH, 128).T)            # [128, 8]
    wo = W_out[:, 0:H] if fwd else W_out[:, H : 2 * H]             # [T, H]
    wout = np.ascontiguousarray(wo.T).astype(ml_dtypes.bfloat16)   # [H, T]

    return {
        "emb": emb_bf,
        "idx": idx_t,
        "wih": wih,
        "whh": whh,
        "bias": bias,
        "wout": wout,
    }


def kernel(**inputs) -> np.ndarray:
    ids = np.asarray(inputs["inputs"]).astype(np.int32)      # [S, B]
    emb_bf = np.asarray(inputs["emb"], np.float32).astype(ml_dtypes.bfloat16)
    weights = [
        (
            np.asarray(inputs["W_ih_f"], np.float32),
            np.asarray(inputs["W_hh_f"], np.float32),
            np.asarray(inputs["b_ih_f"], np.float32),
            np.asarray(inputs["b_hh_f"], np.float32),
            np.asarray(inputs["W_out"], np.float32),
        ),
        (
            np.asarray(inputs["W_ih_b"], np.float32),
            np.asarray(inputs["W_hh_b"], np.float32),
            np.asarray(inputs["b_ih_b"], np.float32),
            np.asarray(inputs["b_hh_b"], np.float32),
            np.asarray(inputs["W_out"], np.float32),
        ),
    ]

    nc = _get_program()
    in_maps = [_core_inputs(k, ids, emb_bf, weights) for k in range(NCORES)]
    import os

    kw = {}
    if os.environ.get("KERNEL_TRACE"):
        kw = {"trace": True, "tmpdir": os.environ.get("KERNEL_TRACE_DIR") or None}
    r = run_bass_kernel_spmd(nc, in_maps, list(range(NCORES)), **kw)
    global LAST_RESULTS
    LAST_RESULTS = r
    res = r.results

    out = np.zeros((S, B, T), np.float32)
    for core in range(NCORES):
        bsl = slice((core % 4) * BL, (core % 4) * BL + BL)
        part = res[core]["out"]                  # [T, NTOK], stream-major
        part = part.reshape(T, 2, S, BL // 2).transpose(2, 1, 3, 0).reshape(S, BL, T)
        if core >= 4:
            part = part[::-1]
        out[:, bsl, :] += part
    out += np.asarray(inputs["b_out"], np.float32)
    return out


# revision 19
# speedup vs baseline: 1.5332x; 1.5332x over previous
"""BiLSTM tagger on 8 Trainium2 NeuronCores.

Reference computation (S=512, B=64, V=100000, E=128, H=256, T=64):
    x  = emb[inputs]                                  # [S,B,E]
    hf = LSTM_f(x);  hb = reverse(LSTM_b(reverse(x))) # [S,B,H] each
    out = concat(hf,hb) @ W_out.T + b_out             # [S,B,T]

Sharding: data-parallel over batch *and* direction.  Cores 0-3 run the
forward LSTM on batch slices of 16; cores 4-7 run the backward LSTM
(time-reversed indices) on the same batch slices.  Each core computes a
partial output projection with its direction's half of W_out; the host
sums fwd+bwd partials and adds b_out.

Per-core device pipeline (all compute on the NeuronCore):
  1. indirect-DMA gather of embedding rows (bf16 table) -> [tok,E] tiles
  2. PE transpose -> xT [E, 8192]
  3. x-projection GEMM (W_ih, bf16) + bias -> xpT [128, S*128] bf16 in SBUF
     (gate rows permuted to chunk order [i0,i1,f0,f1,o0,o1,g0,g1])
  4. 512-step LSTM scan: per step 16 matmuls (W_hh stationary, bf16,
     fast-weight-load) into PSUM, gates on scalar/vector engines in a
     [128, 8*16] packed layout, fp32 cell state, bf16 h
  5. output projection GEMM from saved h history, partials DMA'd out
"""

import sys

for _p in ("/opt/trn_rl_repo",):
    if _p not in sys.path:
        sys.path.insert(0, _p)

import numpy as np
import ml_dtypes

import concourse.bass as bass
import concourse.bacc as bacc
import concourse.mybir as mybir
import concourse.tile as tile
from concourse.bass import ts
from concourse.bass_utils import run_bass_kernel_spmd
from concourse.masks import make_identity

BF16 = mybir.dt.bfloat16
F32 = mybir.dt.float32
AF = mybir.ActivationFunctionType

S, B, V, E, H, T = 512, 64, 100000, 128, 256, 64
NCORES = 8
BL = B // (NCORES // 2)      # 16 batch per core
NTOK = S * BL                # 8192 tokens per core
G4H = 4 * H                  # 1024 gate rows
NCH = G4H // 128             # 8 gate-row chunks
NJT = NTOK // 128            # 64 gather tiles
NSL = NTOK // 512            # 16 GEMM slices

# gate-row permutation: torch order i,f,g,o -> chunk order g,i,f,o.
# g first so its PSUM regions finish first and tanh(g) hides under the
# sigmoid; i,f adjacent for the paired [i|f]*[tanh_g|c] multiply; o last.
_PERM = np.concatenate(
    [
        np.arange(2 * H, 3 * H),   # g
        np.arange(0, H),           # i
        np.arange(H, 2 * H),       # f
        np.arange(3 * H, 4 * H),   # o
    ]
)


def build_program(n_steps: int = S) -> bass.Bass:
    NTOK = n_steps * BL
    NJT = NTOK // 128
    NSL = NTOK // 512

    nc = bacc.Bacc("TRN2", target_bir_lowering=False, debug=False)

    emb_d = nc.declare_dram_parameter("emb", [V, E], BF16, isOutput=False)
    idx_d = nc.declare_dram_parameter("idx", [128, NJT], mybir.dt.int32, isOutput=False)
    wih_d = nc.declare_dram_parameter("wih", [128, G4H], BF16, isOutput=False)
    whh_d = nc.declare_dram_parameter("whh", [H, G4H], BF16, isOutput=False)
    bias_d = nc.declare_dram_parameter("bias", [128, NCH], F32, isOutput=False)
    wout_d = nc.declare_dram_parameter("wout", [H, T], BF16, isOutput=False)
    out_d = nc.declare_dram_parameter("out", [T, NTOK], F32, isOutput=True)

    with tile.TileContext(nc) as tc:
        with (
            tc.tile_pool(name="persist", bufs=1) as pp,
            tc.tile_pool(name="tpsum", bufs=1, space="PSUM") as tpp,
            tc.tile_pool(name="gpsum", bufs=2, space="PSUM") as gpp,
            tc.tile_pool(name="spsum", bufs=2, space="PSUM") as spp,
            tc.tile_pool(name="opsum", bufs=1, space="PSUM") as opp,
            tc.tile_pool(name="gwork", bufs=3) as gwp,
            tc.tile_pool(name="swork", bufs=3) as swp,
        ):
            # ---- persistent SBUF tensors ----
            idx_sb = pp.tile([128, NJT], mybir.dt.int32, tag="idx")
            wih_sb = pp.tile([128, G4H], BF16, tag="wih")
            whh0_sb = pp.tile([128, G4H], BF16, tag="whh0")
            whh1_sb = pp.tile([128, G4H], BF16, tag="whh1")
            bias_sb = pp.tile([128, NCH], F32, tag="bias")
            wout_sb = pp.tile([128, 2 * T], BF16, tag="wout")
            ident = pp.tile([128, 128], BF16, tag="ident")
            xpT = pp.tile([128, n_steps * 128], BF16, tag="xpT")
            # h history: slot t holds [k0-chunk | k1-chunk] of h_t, bf16
            hs = pp.tile([128, (n_steps + 1) * 2 * BL], BF16, tag="hs")
            # [tanh(g) | c] pair; c is persistent in the high half
            gc = pp.tile([128, 4 * BL], F32, tag="gc")
            # flat gather buffer: every gather writes a virgin region (the
            # dynamic-DMA descriptor has a single sem-wait slot, so a gather
            # may carry at most one dependency).  Each [tok,E] region is then
            # transposed in place (PE transpose -> PSUM -> DVE copy back) to
            # [E,tok], and the GEMM reads the buffer directly as rhs.
            xgb = pp.tile([128, NTOK], BF16, tag="xgb")

            # ---- load constants ----
            # idx goes through gpsimd's SWDGE queue (same queue as the
            # indirect gathers) so the gather needs no cross-queue wait:
            # the dynamic-DMA descriptor format only fits one sem wait.
            nc.gpsimd.dma_start(out=idx_sb[:], in_=idx_d[:])
            nc.sync.dma_start(out=wih_sb[:], in_=wih_d[:])
            nc.sync.dma_start(out=whh0_sb[:], in_=whh_d[0:128, :])
            nc.sync.dma_start(out=whh1_sb[:], in_=whh_d[128:256, :])
            nc.sync.dma_start(out=bias_sb[:], in_=bias_d[:])
            nc.sync.dma_start(out=wout_sb[:, 0:T], in_=wout_d[0:128, :])
            nc.sync.dma_start(out=wout_sb[:, T : 2 * T], in_=wout_d[128:256, :])
            make_identity(nc, ident[:])

            nc.gpsimd.memset(hs[:, 0 : 2 * BL], 0.0)
            nc.gpsimd.memset(gc[:], 0.0)

            # ---- gather + in-place transpose + x-projection GEMM ----
            xp4 = xpT[:].rearrange("p (t c b) -> p t c b", c=NCH, b=BL)
            for j in range(NJT):
                nc.gpsimd.indirect_dma_start(
                    out=xgb[:, ts(j, 128)],
                    out_offset=None,
                    in_=emb_d[:],
                    in_offset=bass.IndirectOffsetOnAxis(
                        ap=idx_sb[:, j : j + 1], axis=0
                    ),
                )
                tp = tpp.tile([128, 128], BF16, tag="tp")
                nc.tensor.transpose(tp[:], xgb[:, ts(j, 128)], ident[:])
                # scalar-engine copy: the DVE copy lowers to the one-wait-slot
                # S4D4_TR encoding, and this instruction needs two waits
                # (PSUM ready + gather WAW on the region)
                nc.scalar.copy(out=xgb[:, ts(j, 128)], in_=tp[:])
            for s in range(NSL):
                for c in range(NCH):
                    pg = gpp.tile([128, 512], F32, tag="pg")
                    nc.tensor.matmul(
                        pg[:],
                        lhsT=wih_sb[:, ts(c, 128)],
                        rhs=xgb[:, ts(s, 512)],
                        start=True,
                        stop=True,
                    )
                    # copy+bias into the packed [t, c, b] layout
                    nc.scalar.activation(
                        out=xp4[:, s * 32 : (s + 1) * 32, c, :],
                        in_=pg[:].rearrange("p (t b) -> p t b", b=BL),
                        func=AF.Identity,
                        bias=bias_sb[:, c : c + 1],
                        scale=1.0,
                    )

            # ---- LSTM scan ----
            # Per step: one identity-matmul accumulates the whole xp slice
            # into PSUM, then 16 W_hh matmuls (8 gate chunks x 2 H k-tiles)
            # in chunk order g,i,f,o so tanh(g) can start 4 matmuls in.
            # Gates: sigmoid(i,f) -> t12 = [i|f]*[tanh_g|c] -> c = t12a+t12b
            # -> tanh(c) -> h = sigmoid(o)*tanh(c); sigmoid(o) runs off the
            # critical path.  PSUM cols: g 0:32, i 32:64, f 64:96, o 96:128.
            for t in range(n_steps):
                ps = spp.tile([128, 128], F32, tag="ps")
                nc.tensor.matmul(
                    ps[:],
                    lhsT=ident[:],
                    rhs=xpT[:, ts(t, 128)],
                    start=True,
                    stop=False,
                    skip_group_check=True,
                )
                h0 = hs[:, t * 2 * BL : t * 2 * BL + BL]
                h1 = hs[:, t * 2 * BL + BL : (t + 1) * 2 * BL]
                for c in range(NCH):
                    nc.tensor.matmul(
                        ps[:, ts(c, BL)],
                        lhsT=whh0_sb[:, ts(c, 128)],
                        rhs=h0,
                        start=False,
                        stop=False,
                        skip_group_check=True,
                    )
                    nc.tensor.matmul(
                        ps[:, ts(c, BL)],
                        lhsT=whh1_sb[:, ts(c, 128)],
                        rhs=h1,
                        start=False,
                        stop=True,
                        skip_group_check=True,
                    )
                nc.scalar.activation(gc[:, 0 : 2 * BL], ps[:, 0 : 2 * BL], AF.Tanh)
                gif = gwp.tile([128, 4 * BL], F32, tag="gif")
                nc.scalar.activation(gif[:], ps[:, 2 * BL : 6 * BL], AF.Sigmoid)
                go = gwp.tile([128, 2 * BL], F32, tag="go")
                nc.scalar.activation(go[:], ps[:, 6 * BL : 8 * BL], AF.Sigmoid)
                t12 = swp.tile([128, 4 * BL], F32, tag="t12")
                nc.vector.tensor_mul(t12[:], gif[:], gc[:])
                nc.vector.tensor_add(
                    gc[:, 2 * BL : 4 * BL],
                    t12[:, 0 : 2 * BL],
                    t12[:, 2 * BL : 4 * BL],
                )
                th = swp.tile([128, 2 * BL], F32, tag="th")
                nc.scalar.activation(th[:], gc[:, 2 * BL : 4 * BL], AF.Tanh)
                nc.vector.tensor_mul(
                    hs[:, (t + 1) * 2 * BL : (t + 2) * 2 * BL], go[:], th[:]
                )

            # ---- output projection ----
            hs4 = hs[:].rearrange("p (t k b) -> p t k b", k=2, b=BL)
            for s in range(NSL):
                po = opp.tile([T, 512], F32, tag="po")
                tsl = slice(1 + s * 32, 1 + (s + 1) * 32)
                nc.tensor.matmul(
                    po[:],
                    lhsT=wout_sb[:, 0:T],
                    rhs=hs4[:, tsl, 0, :],
                    start=True,
                    stop=False,
                )
                nc.tensor.matmul(
                    po[:],
                    lhsT=wout_sb[:, T : 2 * T],
                    rhs=hs4[:, tsl, 1, :],
                    start=False,
                    stop=True,
                )
                og = swp.tile([T, 512], F32, tag="og")
                nc.vector.tensor_copy(out=og[:], in_=po[:])
                nc.sync.dma_start(out=out_d[:, ts(s, 512)], in_=og[:])

    nc.compile()
    return nc


_PROGRAM_CACHE: list = []


def _get_program() -> bass.Bass:
    if not _PROGRAM_CACHE:
        _PROGRAM_CACHE.append(build_program())
    return _PROGRAM_CACHE[0]


def _core_inputs(core, inputs_i32, emb_bf, weights):
    fwd = core < 4
    bsl = slice((core % 4) * BL, (core % 4) * BL + BL)
    W_ih, W_hh, b_ih, b_hh, W_out = weights[0 if fwd else 1]

    ids = inputs_i32[:, bsl]
    if not fwd:
        ids = ids[::-1]
    idx_t = np.ascontiguousarray(ids.reshape(NJT, 128).T).astype(np.int32)

    Wihp = W_ih[_PERM]                       # [4H, E]
    wih = np.ascontiguousarray(Wihp.T).astype(ml_dtypes.bfloat16)  # [E, 4H]
    Whhp = W_hh[_PERM]                       # [4H, H]
    whh = np.ascontiguousarray(Whhp.T).astype(ml_dtypes.bfloat16)  # [H, 4H]
    bp = (b_ih + b_hh)[_PERM].astype(np.float32)
    bias = np.ascontiguousarray(bp.reshape(NCH, 128).T)            # [128, 8]
    wo = W_out[:, 0:H] if fwd else W_out[:, H : 2 * H]             # [T, H]
    wout = np.ascontiguousarray(wo.T).astype(ml_dtypes.bfloat16)   # [H, T]

    return {
        "emb": emb_bf,
        "idx": idx_t,
        "wih": wih,
        "whh": whh,
        "bias": bias,
        "wout": wout,
    }


def kernel(**inputs) -> np.ndarray:
    ids = np.asarray(inputs["inputs"]).astype(np.int32)      # [S, B]
    emb_bf = np.asarray(inputs["emb"], np.float32).astype(ml_dtypes.bfloat16)
    weights = [
        (
            np.asarray(inputs["W_ih_f"], np.float32),
            np.asarray(inputs["W_hh_f"], np.float32),
            np.asarray(inputs["b_ih_f"], np.float32),
            np.asarray(inputs["b_hh_f"], np.float32),
            np.asarray(inputs["W_out"], np.float32),
        ),
        (
            np.asarray(inputs["W_ih_b"], np.float32),
            np.asarray(inputs["W_hh_b"], np.float32),
            np.asarray(inputs["b_ih_b"], np.float32),
            np.asarray(inputs["b_hh_b"], np.float32),
            np.asarray(inputs["W_out"], np.float32),
        ),
    ]

    nc = _get_program()
    in_maps = [_core_inputs(k, ids, emb_bf, weights) for k in range(NCORES)]
    import os

    kw = {}
    if os.environ.get("KERNEL_TRACE"):
        kw = {"trace": True, "tmpdir": os.environ.get("KERNEL_TRACE_DIR") or None}
    r = run_bass_kernel_spmd(nc, in_maps, list(range(NCORES)), **kw)
    global LAST_RESULTS
    LAST_RESULTS = r
    res = r.results

    out = np.zeros((S, B, T), np.float32)
    for core in range(NCORES):
        bsl = slice((core % 4) * BL, (core % 4) * BL + BL)
        part = res[core]["out"]                  # [T, NTOK], tokens t-major
        part = part.T.reshape(S, BL, T)
        if core >= 4:
            part = part[::-1]
        out[:, bsl, :] += part
    out += np.asarray(inputs["b_out"], np.float32)
    return out


# revision 20
# speedup vs baseline: 1.8197x; 1.1869x over previous
"""BiLSTM tagger on 8 Trainium2 NeuronCores.

Reference computation (S=512, B=64, V=100000, E=128, H=256, T=64):
    x  = emb[inputs]                                  # [S,B,E]
    hf = LSTM_f(x);  hb = reverse(LSTM_b(reverse(x))) # [S,B,H] each
    out = concat(hf,hb) @ W_out.T + b_out             # [S,B,T]

Sharding: data-parallel over batch *and* direction.  Cores 0-3 run the
forward LSTM on batch slices of 16; cores 4-7 run the backward LSTM
(time-reversed indices) on the same batch slices.  Each core computes a
partial output projection with its direction's half of W_out; the host
sums fwd+bwd partials and adds b_out.

Per-core device pipeline (all compute on the NeuronCore):
  1. indirect-DMA gather of embedding rows (bf16 table) -> [tok,E] tiles
  2. PE transpose -> xT [E, 8192]
  3. x-projection GEMM (W_ih, bf16) + bias -> xpT [128, S*128] bf16 in SBUF
     (gate rows permuted to chunk order [i0,i1,f0,f1,o0,o1,g0,g1])
  4. 512-step LSTM scan: per step 16 matmuls (W_hh stationary, bf16,
     fast-weight-load) into PSUM, gates on scalar/vector engines in a
     [128, 8*16] packed layout, fp32 cell state, bf16 h
  5. output projection GEMM from saved h history, partials DMA'd out
"""

import sys

for _p in ("/opt/trn_rl_repo",):
    if _p not in sys.path:
        sys.path.insert(0, _p)

import numpy as np
import ml_dtypes

import concourse.bass as bass
import concourse.bacc as bacc
import concourse.mybir as mybir
import concourse.tile as tile
from concourse.bass import ts
from concourse.bass_utils import run_bass_kernel_spmd
from concourse.masks import make_identity

BF16 = mybir.dt.bfloat16
F32 = mybir.dt.float32
AF = mybir.ActivationFunctionType

S, B, V, E, H, T = 512, 64, 100000, 128, 256, 64
NCORES = 8
BL = B // (NCORES // 2)      # 16 batch per core
NTOK = S * BL                # 8192 tokens per core
G4H = 4 * H                  # 1024 gate rows
NCH = G4H // 128             # 8 gate-row chunks
NJT = NTOK // 128            # 64 gather tiles
NSL = NTOK // 512            # 16 GEMM slices

# gate-row permutation: torch order i,f,g,o -> chunk order g,i,f,o.
# g first so its PSUM regions finish first and tanh(g) hides under the
# sigmoid; i,f adjacent for the paired [i|f]*[tanh_g|c] multiply; o last.
_PERM = np.concatenate(
    [
        np.arange(2 * H, 3 * H),   # g
        np.arange(0, H),           # i
        np.arange(H, 2 * H),       # f
        np.arange(3 * H, 4 * H),   # o
    ]
)


def build_program(n_steps: int = S) -> bass.Bass:
    NTOK = n_steps * BL
    NJT = NTOK // 128
    NSL = NTOK // 512

    nc = bacc.Bacc("TRN2", target_bir_lowering=False, debug=False)

    emb_d = nc.declare_dram_parameter("emb", [V, E], BF16, isOutput=False)
    idx_d = nc.declare_dram_parameter("idx", [128, NJT], mybir.dt.int32, isOutput=False)
    wih_d = nc.declare_dram_parameter("wih", [128, G4H], BF16, isOutput=False)
    whh_d = nc.declare_dram_parameter("whh", [H, G4H], BF16, isOutput=False)
    bias_d = nc.declare_dram_parameter("bias", [128, NCH], F32, isOutput=False)
    wout_d = nc.declare_dram_parameter("wout", [H, T], BF16, isOutput=False)
    out_d = nc.declare_dram_parameter("out", [T, NTOK], F32, isOutput=True)

    with tile.TileContext(nc) as tc:
        with (
            tc.tile_pool(name="persist", bufs=1) as pp,
            tc.tile_pool(name="tpsum", bufs=1, space="PSUM") as tpp,
            tc.tile_pool(name="gpsum", bufs=2, space="PSUM") as gpp,
            tc.tile_pool(name="spsumg", bufs=1, space="PSUM") as spg,
            tc.tile_pool(name="spsumi", bufs=1, space="PSUM") as spi,
            tc.tile_pool(name="spsumo", bufs=1, space="PSUM") as spo,
            tc.tile_pool(name="opsum", bufs=1, space="PSUM") as opp,
            tc.tile_pool(name="gwork", bufs=3) as gwp,
            tc.tile_pool(name="swork", bufs=3) as swp,
        ):
            # ---- persistent SBUF tensors ----
            idx_sb = pp.tile([128, NJT], mybir.dt.int32, tag="idx")
            wih_sb = pp.tile([128, G4H], BF16, tag="wih")
            whh0_sb = pp.tile([128, G4H], BF16, tag="whh0")
            whh1_sb = pp.tile([128, G4H], BF16, tag="whh1")
            bias_sb = pp.tile([128, NCH], F32, tag="bias")
            wout_sb = pp.tile([128, 2 * T], BF16, tag="wout")
            ident = pp.tile([128, 128], BF16, tag="ident")
            xpT = pp.tile([128, n_steps * 128], BF16, tag="xpT")
            # h history: slot t holds [k0-chunk | k1-chunk] of h_t, bf16
            hs = pp.tile([128, (n_steps + 1) * 2 * BL], BF16, tag="hs")
            # [tanh(g) | c] pair; c is persistent in the high half
            gc = pp.tile([128, 4 * BL], F32, tag="gc")
            # flat gather buffer: every gather writes a virgin region (the
            # dynamic-DMA descriptor has a single sem-wait slot, so a gather
            # may carry at most one dependency).  Each [tok,E] region is then
            # transposed in place (PE transpose -> PSUM -> DVE copy back) to
            # [E,tok], and the GEMM reads the buffer directly as rhs.
            xgb = pp.tile([128, NTOK], BF16, tag="xgb")

            # ---- load constants ----
            # idx goes through gpsimd's SWDGE queue (same queue as the
            # indirect gathers) so the gather needs no cross-queue wait:
            # the dynamic-DMA descriptor format only fits one sem wait.
            nc.gpsimd.dma_start(out=idx_sb[:], in_=idx_d[:])
            nc.sync.dma_start(out=wih_sb[:], in_=wih_d[:])
            nc.sync.dma_start(out=whh0_sb[:], in_=whh_d[0:128, :])
            nc.sync.dma_start(out=whh1_sb[:], in_=whh_d[128:256, :])
            nc.sync.dma_start(out=bias_sb[:], in_=bias_d[:])
            nc.sync.dma_start(out=wout_sb[:, 0:T], in_=wout_d[0:128, :])
            nc.sync.dma_start(out=wout_sb[:, T : 2 * T], in_=wout_d[128:256, :])
            make_identity(nc, ident[:])

            nc.gpsimd.memset(hs[:, 0 : 2 * BL], 0.0)
            nc.gpsimd.memset(gc[:], 0.0)

            # ---- gather + in-place transpose + x-projection GEMM ----
            xp4 = xpT[:].rearrange("p (t c b) -> p t c b", c=NCH, b=BL)
            for j in range(NJT):
                nc.gpsimd.indirect_dma_start(
                    out=xgb[:, ts(j, 128)],
                    out_offset=None,
                    in_=emb_d[:],
                    in_offset=bass.IndirectOffsetOnAxis(
                        ap=idx_sb[:, j : j + 1], axis=0
                    ),
                )
                tp = tpp.tile([128, 128], BF16, tag="tp")
                nc.tensor.transpose(tp[:], xgb[:, ts(j, 128)], ident[:])
                # scalar-engine copy: the DVE copy lowers to the one-wait-slot
                # S4D4_TR encoding, and this instruction needs two waits
                # (PSUM ready + gather WAW on the region)
                nc.scalar.copy(out=xgb[:, ts(j, 128)], in_=tp[:])
            for s in range(NSL):
                for c in range(NCH):
                    pg = gpp.tile([128, 512], F32, tag="pg")
                    nc.tensor.matmul(
                        pg[:],
                        lhsT=wih_sb[:, ts(c, 128)],
                        rhs=xgb[:, ts(s, 512)],
                        start=True,
                        stop=True,
                    )
                    # copy+bias into the packed [t, c, b] layout
                    nc.scalar.activation(
                        out=xp4[:, s * 32 : (s + 1) * 32, c, :],
                        in_=pg[:].rearrange("p (t b) -> p t b", b=BL),
                        func=AF.Identity,
                        bias=bias_sb[:, c : c + 1],
                        scale=1.0,
                    )

            # ---- LSTM scan ----
            # Three PSUM tiles (g / if / o) so each gate activation only
            # waits for its own accumulation group, not the whole burst.
            # W matmuls run g-chunks first; ACT order tanh(g), sigmoid(i,f)
            # [chain], sigmoid(o) [off-chain].  The identity-matmuls that
            # seed PSUM with xp have no h dependence and execute during the
            # previous step's gate window.  Single-buffered PSUM: the seed
            # matmul's WAR stall on last step's gate reads resolves inside
            # that window.
            for t in range(n_steps):
                psg = spg.tile([128, 2 * BL], F32, tag="psg")
                psi = spi.tile([128, 4 * BL], F32, tag="psi")
                pso = spo.tile([128, 2 * BL], F32, tag="pso")
                xps = xpT[:, ts(t, 128)]
                nc.tensor.matmul(
                    psg[:], lhsT=ident[:], rhs=xps[:, 0 : 2 * BL],
                    start=True, stop=False, skip_group_check=True,
                )
                nc.tensor.matmul(
                    psi[:], lhsT=ident[:], rhs=xps[:, 2 * BL : 6 * BL],
                    start=True, stop=False, skip_group_check=True,
                )
                nc.tensor.matmul(
                    pso[:], lhsT=ident[:], rhs=xps[:, 6 * BL : 8 * BL],
                    start=True, stop=False, skip_group_check=True,
                )
                h0 = hs[:, t * 2 * BL : t * 2 * BL + BL]
                h1 = hs[:, t * 2 * BL + BL : (t + 1) * 2 * BL]
                for cc, tile_, base in (
                    (0, psg, 0), (1, psg, 0),
                    (2, psi, 2), (3, psi, 2), (4, psi, 2), (5, psi, 2),
                    (6, pso, 6), (7, pso, 6),
                ):
                    reg = tile_[:, (cc - base) * BL : (cc - base + 1) * BL]
                    nc.tensor.matmul(
                        reg, lhsT=whh0_sb[:, ts(cc, 128)], rhs=h0,
                        start=False, stop=False, skip_group_check=True,
                    )
                    nc.tensor.matmul(
                        reg, lhsT=whh1_sb[:, ts(cc, 128)], rhs=h1,
                        start=False, stop=True, skip_group_check=True,
                    )
                nc.scalar.activation(gc[:, 0 : 2 * BL], psg[:], AF.Tanh)
                gif = gwp.tile([128, 4 * BL], F32, tag="gif")
                nc.scalar.activation(gif[:], psi[:], AF.Sigmoid)
                go = gwp.tile([128, 2 * BL], F32, tag="go")
                nc.scalar.activation(go[:], pso[:], AF.Sigmoid)
                t12 = swp.tile([128, 4 * BL], F32, tag="t12")
                nc.vector.tensor_mul(t12[:], gif[:], gc[:])
                nc.vector.tensor_add(
                    gc[:, 2 * BL : 4 * BL],
                    t12[:, 0 : 2 * BL],
                    t12[:, 2 * BL : 4 * BL],
                )
                th = swp.tile([128, 2 * BL], F32, tag="th")
                nc.scalar.activation(th[:], gc[:, 2 * BL : 4 * BL], AF.Tanh)
                nc.vector.tensor_mul(
                    hs[:, (t + 1) * 2 * BL : (t + 2) * 2 * BL], go[:], th[:]
                )

            # ---- output projection ----
            hs4 = hs[:].rearrange("p (t k b) -> p t k b", k=2, b=BL)
            for s in range(NSL):
                po = opp.tile([T, 512], F32, tag="po")
                tsl = slice(1 + s * 32, 1 + (s + 1) * 32)
                nc.tensor.matmul(
                    po[:],
                    lhsT=wout_sb[:, 0:T],
                    rhs=hs4[:, tsl, 0, :],
                    start=True,
                    stop=False,
                )
                nc.tensor.matmul(
                    po[:],
                    lhsT=wout_sb[:, T : 2 * T],
                    rhs=hs4[:, tsl, 1, :],
                    start=False,
                    stop=True,
                )
                og = swp.tile([T, 512], F32, tag="og")
                nc.vector.tensor_copy(out=og[:], in_=po[:])
                nc.sync.dma_start(out=out_d[:, ts(s, 512)], in_=og[:])

    nc.compile()
    return nc


_PROGRAM_CACHE: list = []


def _get_program() -> bass.Bass:
    if not _PROGRAM_CACHE:
        _PROGRAM_CACHE.append(build_program())
    return _PROGRAM_CACHE[0]


def _core_inputs(core, inputs_i32, emb_bf, weights):
    fwd = core < 4
    bsl = slice((core % 4) * BL, (core % 4) * BL + BL)
    W_ih, W_hh, b_ih, b_hh, W_out = weights[0 if fwd else 1]

    ids = inputs_i32[:, bsl]
    if not fwd:
        ids = ids[::-1]
    idx_t = np.ascontiguousarray(ids.reshape(NJT, 128).T).astype(np.int32)

    Wihp = W_ih[_PERM]                       # [4H, E]
    wih = np.ascontiguousarray(Wihp.T).astype(ml_dtypes.bfloat16)  # [E, 4H]
    Whhp = W_hh[_PERM]                       # [4H, H]
    whh = np.ascontiguousarray(Whhp.T).astype(ml_dtypes.bfloat16)  # [H, 4H]
    bp = (b_ih + b_hh)[_PERM].astype(np.float32)
    bias = np.ascontiguousarray(bp.reshape(NCH, 128).T)            # [128, 8]
    wo = W_out[:, 0:H] if fwd else W_out[:, H : 2 * H]             # [T, H]
    wout = np.ascontiguousarray(wo.T).astype(ml_dtypes.bfloat16)   # [H, T]

    return {
        "emb": emb_bf,
        "idx": idx_t,
        "wih": wih,
        "whh": whh,
        "bias": bias,
        "wout": wout,
    }


def kernel(**inputs) -> np.ndarray:
    ids = np.asarray(inputs["inputs"]).astype(np.int32)      # [S, B]
    emb_bf = np.asarray(inputs["emb"], np.float32).astype(ml_dtypes.bfloat16)
    weights = [
        (
            np.asarray(inputs["W_ih_f"], np.float32),
            np.asarray(inputs["W_hh_f"], np.float32),
            np.asarray(inputs["b_ih_f"], np.float32),
            np.asarray(inputs["b_hh_f"], np.float32),
            np.asarray(inputs["W_out"], np.float32),
        ),
        (
            np.asarray(inputs["W_ih_b"], np.float32),
            np.asarray(inputs["W_hh_b"], np.float32),
            np.asarray(inputs["b_ih_b"], np.float32),
            np.asarray(inputs["b_hh_b"], np.float32),
            np.asarray(inputs["W_out"], np.float32),
        ),
    ]

    nc = _get_program()
    in_maps = [_core_inputs(k, ids, emb_bf, weights) for k in range(NCORES)]
    import os

    kw = {}
    if os.environ.get("KERNEL_TRACE"):
        kw = {"trace": True, "tmpdir": os.environ.get("KERNEL_TRACE_DIR") or None}
    r = run_bass_kernel_spmd(nc, in_maps, list(range(NCORES)), **kw)
    global LAST_RESULTS
    LAST_RESULTS = r
    res = r.results

    out = np.zeros((S, B, T), np.float32)
    for core in range(NCORES):
        bsl = slice((core % 4) * BL, (core % 4) * BL + BL)
        part = res[core]["out"]                  # [T, NTOK], tokens t-major
        part = part.T.reshape(S, BL, T)
        if core >= 4:
            part = part[::-1]
        out[:, bsl, :] += part
    out += np.asarray(inputs["b_out"], np.float32)
    return out
